# revision 38
# baseline (speedup 1.0000x reference)
"""DSS Linear+BN segment-reduce kernel for Trainium2, 8 NeuronCores.

Problem (N=131072, D=1024, B=2048):
    z_i = BN(x @ W_fc.T + b_fc)                      # per-element path
    x_m = segment_sum(x, seg_ids, B)                 # ragged segment sums
    x_s = BN(x_m @ W_sh.T + b_sh)                    # set path
    out = x_s[seg_ids] + z_i

Strategy (segment-aligned data parallel, column-panel pipeline):
  - Host shards rows by whole segments (256 segs/core, greedily balanced),
    zero-padding each segment to a multiple of 8 rows; biases are absorbed
    by the BN shift so b_fc/b_sh are never used.
  - Output columns are split into 2 panels of 512. Panel 0: matmul sweep
    (z0 spilled bf16 to DRAM, blocked layout), chunk-8 partial sums of x^T
    (PE-transposed to a DRAM chunk table), then the segment gather-reduce,
    set-path matmul and BN-stats AllGather #0.
  - Panel 1's matmul sweep hides panel 0's stats collective, the BN affine,
    and panel 0's output pass: out0 = z0*s_fc + C0 @ onehot(seg), with the
    one-hot built by is_equal and applied by the PE against the 128-seg
    half each block actually touches. z1 stays resident in SBUF.
  - Tail: AllGather #1 + panel-1 output pass (z1 from SBUF, no DMA read).
  - out is written bf16 (blocked layout); host transposes/gathers/upcasts.
"""
import sys
import numpy as np
from contextlib import ExitStack

sys.path.insert(0, "/opt/trn_rl_repo")

import concourse.bass as bass
import concourse.bacc as bacc
import concourse.tile as tile
from concourse import mybir
from concourse.bass_utils import run_bass_kernel_spmd

F32 = mybir.dt.float32
BF16 = mybir.dt.bfloat16
I32 = mybir.dt.int32
AX = mybir.AxisListType.X
ALU = mybir.AluOpType

N, D, B, NC = 131072, 1024, 2048, 8
B_PER = B // NC            # 256 segments per core
EPS = 1e-5
CH = 8                     # segment padding / chunk size
RB = 512                   # rows per block (matmul free dim)
KC = D // 128              # 8 k-chunks
NP = 2                     # output column panels
PW = D // NP               # 512 cols per panel
DCP = PW // 128            # 4 d-chunks per panel
Z1SP = 13                   # panel-1 z blocks spilled to DRAM (SBUF relief)

_cache = {}


def _plan(seg_ids):
    """Host planning: per-core padded layouts + gather indices + block
    half-windows (which 128-seg half each 512-row block touches, unioned
    across cores so the SPMD program is uniform)."""
    seg_ids = np.asarray(seg_ids)
    counts = np.bincount(seg_ids, minlength=B).astype(np.int64)
    row_start = np.zeros(B + 1, dtype=np.int64)
    np.cumsum(counts, out=row_start[1:])

    pad = ((counts + CH - 1) // CH) * CH          # padded len per segment
    order = np.argsort(-pad, kind="stable")
    load = np.zeros(NC, dtype=np.int64)
    nseg = np.zeros(NC, dtype=np.int64)
    assign = np.empty(B, dtype=np.int64)
    for b in order:
        cands = np.where(nseg < B_PER)[0]
        c = cands[np.argmin(load[cands])]
        assign[b] = c
        load[c] += pad[b]
        nseg[c] += 1
    max_rows = int(((load.max() + 2 * RB - 1) // (2 * RB)) * (2 * RB))
    nblk = max_rows // RB
    nchunk = max_rows // CH
    slots = int((pad // CH).max())
    assert slots >= 1

    plans = []
    halves_per_core = []
    for c in range(NC):
        segs = np.where(assign == c)[0]          # global segment ids, sorted
        cnt = counts[segs]
        pd = pad[segs]
        pstart = np.zeros(B_PER, dtype=np.int64)
        np.cumsum(pd[:-1], out=pstart[1:])
        nreal = int(cnt.sum())
        gr = np.concatenate(
            [np.arange(row_start[b], row_start[b + 1]) for b in segs]) \
            if nreal else np.empty(0, dtype=np.int64)
        local_b = np.repeat(np.arange(B_PER), cnt)
        col_ids = np.repeat(pstart, cnt) + \
            (np.arange(nreal) - np.repeat(np.cumsum(cnt) - cnt, cnt))
        nch = (pd // CH).astype(np.int64)
        cstart = pstart // CH
        idx = np.full((128, 2 * slots), nchunk, dtype=np.int32)  # zero row
        for h in range(2):
            b_loc = h * 128 + np.arange(128)
            for j in range(slots):
                m = nch[b_loc] > j
                idx[m, h * slots + j] = (cstart[b_loc] + j)[m]
        # which half each block touches
        seg_of_col = np.full(max_rows, -1, dtype=np.int64)
        seg_of_col[col_ids] = local_b
        hs = []
        for jb in range(nblk):
            v = seg_of_col[jb * RB:(jb + 1) * RB]
            v = v[v >= 0]
            hs.append(frozenset((v // 128).tolist()) if v.size else frozenset())
        halves_per_core.append(hs)
        plans.append(dict(
            grows=gr,
            nreal=nreal,
            col_ids=col_ids,
            local_b=local_b,
            idx=idx,
        ))
    halves = []
    for jb in range(nblk):
        u = frozenset().union(*[halves_per_core[c][jb] for c in range(NC)])
        halves.append(tuple(sorted(u)) if u else (1,))
    return counts, plans, max_rows, slots, tuple(halves)


def _build(max_rows, slots, halves):
    nblk = max_rows // RB
    nchunk = max_rows // CH
    npair = nblk // 2

    nc = bacc.Bacc("TRN2", target_bir_lowering=False, debug=False,
                   num_devices=NC)

    xB = nc.dram_tensor("xB", [nblk, 128, KC, RB], BF16,
                        kind="ExternalInput").ap()
    sid = nc.dram_tensor("sid", [1, max_rows], BF16, kind="ExternalInput").ap()
    idx = nc.dram_tensor("idx", [128, 2 * slots], I32, kind="ExternalInput").ap()
    wfT = nc.dram_tensor("wfT", [D, D], BF16, kind="ExternalInput").ap()
    wsT = nc.dram_tensor("wsT", [D, D], BF16, kind="ExternalInput").ap()
    # params [128, 162]: 0:8 g_fc, 8:16 be_fc, 16:24 g_sh, 24:32 be_sh,
    #                    32:34 iota halves, 34:162 identity
    par = nc.dram_tensor("par", [128, 162], F32, kind="ExternalInput").ap()
    outs = [nc.dram_tensor(f"out{p}", [nblk, 128, DCP, RB], BF16,
                           kind="ExternalOutput").ap() for p in range(NP)]

    wfT3 = wfT.rearrange("(kc p) d -> p kc d", p=128)
    wsT3 = wsT.rearrange("(kc p) d -> p kc d", p=128)

    with tile.TileContext(nc) as tc:
        with ExitStack() as top:
            keep = top.enter_context(tc.tile_pool(name="keep", bufs=1))
            dram = top.enter_context(tc.tile_pool(name="dram", bufs=1,
                                                  space="DRAM"))

            z0d = dram.tile([nblk, 128, DCP * RB], BF16)
            z1d = dram.tile([Z1SP, 128, DCP * RB], BF16)
            s8t = dram.tile([nchunk + 1, D], BF16)
            d_in = [dram.tile([128, 16], F32, tag=f"din{p}", name=f"din{p}") for p in range(NP)]
            d_ag = [dram.tile([NC, 128, 16], F32, tag=f"dag{p}", name=f"dag{p}") for p in range(NP)]

            p_par = keep.tile([128, 162], F32)
            nc.sync.dma_start(p_par[:], par[:])
            p_idx = keep.tile([128, 2 * slots], I32)
            nc.sync.dma_start(p_idx[:], idx[:])
            ident = p_par[:, 34:162]

            zrow = keep.tile([1, D], BF16)
            nc.vector.memset(zrow[:], 0.0)
            nc.sync.dma_start(s8t[nchunk:nchunk + 1, :], zrow[:])

            bn_el = [keep.tile([128, DCP, nblk, 6], F32, tag=f"bnel{p}", name=f"bnel{p}") for p in range(NP)]
            zres1 = keep.tile([128, nblk - Z1SP, DCP, RB], BF16)  # panel-1 z
            xmT = keep.tile([128, KC, 2 * 128], BF16)
            zs = [keep.tile([128, DCP, 2 * 128], BF16, tag=f"zs{p}", name=f"zs{p}") for p in range(NP)]
            bn_st = [keep.tile([128, DCP, 1, 6], F32, tag=f"bnst{p}", name=f"bnst{p}") for p in range(NP)]
            cn = [keep.tile([128, 2, DCP, 128], BF16, tag=f"cn{p}", name=f"cn{p}") for p in range(NP)]
            s_fc = [keep.tile([128, DCP], F32, tag=f"sfc{p}", name=f"sfc{p}") for p in range(NP)]
            xm = [keep.tile([128, D], F32, tag=f"xm{h}", name=f"xm{h}") for h in range(2)]

            gpool = top.enter_context(tc.tile_pool(name="g", bufs=3))

            def gather_steps():
                steps = []
                for j in range(slots):
                    for h in range(2):
                        def step(j=j, h=h):
                            g = gpool.tile([128, D], BF16, tag=f"g{h}",
                                           name="g")
                            nc.gpsimd.indirect_dma_start(
                                out=g[:],
                                out_offset=None,
                                in_=s8t[:, :],
                                in_offset=bass.IndirectOffsetOnAxis(
                                    ap=p_idx[:, h * slots + j:
                                             h * slots + j + 1],
                                    axis=0),
                                element_offset=0)
                            if j == 0:
                                nc.vector.tensor_copy(xm[h][:], g[:])
                            else:
                                nc.vector.tensor_add(xm[h][:], xm[h][:], g[:])
                        steps.append(step)
                return steps

            # ===================== PANEL 0 SWEEP =====================
            with ExitStack() as pa:
                w0pool = pa.enter_context(tc.tile_pool(name="w0", bufs=1))
                xpool = pa.enter_context(tc.tile_pool(name="xa", bufs=2))
                zpool = pa.enter_context(tc.tile_pool(name="za", bufs=2))
                spool = pa.enter_context(tc.tile_pool(name="sa", bufs=2))
                psA = pa.enter_context(tc.tile_pool(name="psA", bufs=3,
                                                    space="PSUM"))
                psT = pa.enter_context(tc.tile_pool(name="psT", bufs=2,
                                                    space="PSUM"))

                wf0 = w0pool.tile([128, KC, PW], BF16)
                nc.sync.dma_start(wf0[:], wfT3[:, :, 0:PW])

                s8p = None
                for ib in range(nblk):
                    xt = xpool.tile([128, KC, RB], BF16, tag="xt")
                    nc.sync.dma_start(xt[:], xB[ib])
                    zst = zpool.tile([128, DCP, RB], BF16, tag="zst")
                    for dc in range(DCP):
                        pz = psA.tile([128, RB], F32, tag="mm")
                        for kc in range(KC):
                            nc.tensor.matmul(
                                pz[:], wf0[:, kc, dc * 128:(dc + 1) * 128],
                                xt[:, kc, :], start=(kc == 0),
                                stop=(kc == KC - 1))
                        nc.scalar.copy(zst[:, dc, :], pz[:])
                        nc.vector.bn_stats(bn_el[0][:, dc, ib, :],
                                           zst[:, dc, :])
                    nc.sync.dma_start(
                        z0d[ib].rearrange("p (dc r) -> p dc r", dc=DCP),
                        zst[:])

                    # chunk-8 partial sums, paired into 128-chunk groups
                    if ib % 2 == 0:
                        s8p = spool.tile([128, KC, 128], F32, tag="s8p")
                    off = (ib % 2) * (RB // CH)
                    nc.vector.reduce_sum(
                        out=s8p[:, :, off:off + RB // CH],
                        in_=xt[:].rearrange("p kc (c k) -> p kc c k", k=CH),
                        axis=AX)
                    if ib % 2 == 1:
                        s8s = spool.tile([128, KC, 128], F32, tag="s8s")
                        for kc in range(KC):
                            pt = psT.tile([128, 128], F32, tag="tr")
                            nc.tensor.transpose(pt[:], s8p[:, kc, :], ident)
                            nc.vector.tensor_copy(s8s[:, kc, :], pt[:])
                        pr = ib // 2
                        nc.sync.dma_start(
                            s8t[pr * 128:(pr + 1) * 128, :]
                            .rearrange("c (kc k) -> c kc k", kc=KC),
                            s8s[:])

            # ====== PANEL 1 SWEEP (hides stats #0, affine #0, OH0) ======
            with ExitStack() as pb:
                w1pool = pb.enter_context(tc.tile_pool(name="w1", bufs=1))
                xpool = pb.enter_context(tc.tile_pool(name="xb", bufs=2))
                zbpool = pb.enter_context(tc.tile_pool(name="zb", bufs=3))
                epool = pb.enter_context(tc.tile_pool(name="e", bufs=2))
                opool = pb.enter_context(tc.tile_pool(name="o", bufs=2))
                mpool = pb.enter_context(tc.tile_pool(name="mid", bufs=1))
                psA = pb.enter_context(tc.tile_pool(name="psB", bufs=2,
                                                    space="PSUM"))
                psT = pb.enter_context(tc.tile_pool(name="psU", bufs=1,
                                                    space="PSUM"))
                psX = pb.enter_context(tc.tile_pool(name="psX", bufs=3,
                                                    space="PSUM"))

                wf1 = w1pool.tile([128, KC, PW], BF16)
                nc.sync.dma_start(wf1[:], wfT3[:, :, PW:D])
                ws0 = w1pool.tile([128, KC, PW], BF16, tag="ws", name="ws0",
                                  bufs=2)
                nc.sync.dma_start(ws0[:], wsT3[:, :, 0:PW])
                ws1 = w1pool.tile([128, KC, PW], BF16, tag="ws", name="ws1",
                                  bufs=2)
                nc.sync.dma_start(ws1[:], wsT3[:, :, PW:D])

                def p1_block(ib):
                    xt = xpool.tile([128, KC, RB], BF16, tag="xt")
                    nc.sync.dma_start(xt[:], xB[ib])
                    zst = zbpool.tile([128, DCP, RB], BF16, tag="zb") \
                        if ib < Z1SP else None
                    for dc in range(DCP):
                        pz = psA.tile([128, RB], F32, tag="mm")
                        for kc in range(KC):
                            nc.tensor.matmul(
                                pz[:], wf1[:, kc, dc * 128:(dc + 1) * 128],
                                xt[:, kc, :], start=(kc == 0),
                                stop=(kc == KC - 1))
                        zdst = zst[:, dc, :] if ib < Z1SP \
                            else zres1[:, ib - Z1SP, dc, :]
                        nc.scalar.copy(zdst, pz[:])
                        nc.vector.bn_stats(bn_el[1][:, dc, ib, :], zdst)
                    if ib < Z1SP:
                        nc.sync.dma_start(
                            z1d[ib].rearrange("p (dc r) -> p dc r", dc=DCP),
                            zst[:])

                def set_path(pan, ws):
                    for dc in range(DCP):
                        pzs = psT.tile([128, 2 * 128], F32, tag="set")
                        for kc in range(KC):
                            nc.tensor.matmul(
                                pzs[:], ws[:, kc, dc * 128:(dc + 1) * 128],
                                xmT[:, kc, :], start=(kc == 0),
                                stop=(kc == KC - 1))
                        nc.vector.bn_stats(bn_st[pan][:, dc, 0, :], pzs[:])
                        nc.vector.tensor_copy(zs[pan][:, dc, :], pzs[:])

                def pack_stats(pan):
                    loc = mpool.tile([128, 16], F32, tag="loc")
                    mv_i = mpool.tile([128, DCP, 2], F32, tag="mvi")
                    mv_s = mpool.tile([128, DCP, 2], F32, tag="mvs")
                    for dc in range(DCP):
                        nc.vector.bn_aggr(mv_i[:, dc, :], bn_el[pan][:, dc, :, :])
                        nc.vector.bn_aggr(mv_s[:, dc, :], bn_st[pan][:, dc, :, :])
                    tmp = mpool.tile([128, DCP], F32, tag="tm")
                    for (mv, cnt_, o_s, o_q) in ((mv_i, float(max_rows), 0, 4),
                                                 (mv_s, 256.0, 8, 12)):
                        nc.vector.tensor_scalar_mul(
                            loc[:, o_s:o_s + DCP], mv[:, :, 0], cnt_)
                        nc.vector.tensor_mul(tmp[:], mv[:, :, 0], mv[:, :, 0])
                        nc.vector.tensor_add(tmp[:], tmp[:], mv[:, :, 1])
                        nc.vector.tensor_scalar_mul(
                            loc[:, o_q:o_q + DCP], tmp[:], cnt_)
                    nc.sync.dma_start(d_in[pan][:], loc[:])
                    nc.gpsimd.collective_compute(
                        "AllGather", ALU.bypass,
                        replica_groups=[list(range(NC))],
                        ins=[d_in[pan][:].opt()], outs=[d_ag[pan][:].opt()])

                def affine(pan):
                    rk = mpool.tile([128, NC, 16], F32, tag="rk")
                    nc.sync.dma_start(rk[:], d_ag[pan].rearrange("r p j -> p r j"))
                    g16 = mpool.tile([128, 16], F32, tag="g16")
                    nc.vector.reduce_sum(
                        out=g16[:], in_=rk[:].rearrange("p r j -> p j r"),
                        axis=AX)
                    po = pan * DCP

                    def bn_affine(sum_sl, sq_sl, inv_n, g_sl, be_sl, s_out, sfx):
                        m = mpool.tile([128, DCP], F32, tag=f"m{sfx}")
                        nc.vector.tensor_scalar_mul(m[:], g16[:, sum_sl], inv_n)
                        v = mpool.tile([128, DCP], F32, tag=f"v{sfx}")
                        nc.vector.tensor_scalar_mul(v[:], g16[:, sq_sl], inv_n)
                        t2 = mpool.tile([128, DCP], F32, tag=f"t2{sfx}")
                        nc.vector.tensor_mul(t2[:], m[:], m[:])
                        nc.vector.tensor_sub(v[:], v[:], t2[:])
                        nc.vector.tensor_scalar_add(v[:], v[:], EPS)
                        nc.scalar.sqrt(v[:], v[:])
                        nc.vector.reciprocal(v[:], v[:])
                        nc.vector.tensor_mul(s_out[:], v[:],
                                             p_par[:, g_sl])
                        t_out = mpool.tile([128, DCP], F32, tag=f"t{sfx}")
                        nc.vector.tensor_mul(t_out[:], m[:], s_out[:])
                        nc.vector.tensor_sub(t_out[:], p_par[:, be_sl], t_out[:])
                        return t_out

                    t_fc = bn_affine(slice(0, 4), slice(4, 8), 1.0 / N,
                                     slice(po, po + 4), slice(8 + po, 8 + po + 4),
                                     s_fc[pan], "i")
                    s_sh = mpool.tile([128, DCP], F32, tag="ssh")
                    t_sh = bn_affine(slice(8, 12), slice(12, 16), 1.0 / B,
                                     slice(16 + po, 16 + po + 4),
                                     slice(24 + po, 24 + po + 4), s_sh, "s")
                    tb = mpool.tile([128, DCP], F32, tag="tb")
                    nc.vector.tensor_add(tb[:], t_sh[:], t_fc[:])
                    ct = mpool.tile([128, DCP, 2 * 128], F32, tag="ct")
                    for dc in range(DCP):
                        nc.vector.tensor_scalar(
                            out=ct[:, dc, :], in0=zs[pan][:, dc, :],
                            scalar1=s_sh[:, dc:dc + 1], scalar2=tb[:, dc:dc + 1],
                            op0=ALU.mult, op1=ALU.add)
                    for h in range(2):
                        for dc in range(DCP):
                            pt = psT.tile([128, 128], F32, tag="tr")
                            nc.tensor.transpose(
                                pt[:], ct[:, dc, h * 128:(h + 1) * 128], ident)
                            nc.vector.tensor_copy(cn[pan][:, h, dc, :], pt[:])

                GRP = 2

                def oh_group_e(gb):
                    lo = gb * GRP
                    hi = min(lo + GRP, nblk)
                    hs_u = sorted(set().union(
                        *[set(halves[jb]) for jb in range(lo, hi)]))
                    w = (hi - lo) * RB
                    sid4 = epool.tile([128, GRP * RB], BF16, tag="sid4",
                                      name="sid4")
                    nc.sync.dma_start(
                        sid4[:, :w],
                        sid[:1, lo * RB:hi * RB].to_broadcast([128, w]))
                    e4 = {}
                    for h in hs_u:
                        e = epool.tile([128, GRP * RB], BF16, tag=f"e4h{h}",
                                       name="e4h")
                        nc.vector.tensor_tensor(
                            out=e[:, :w],
                            in0=p_par[:, 32 + h:33 + h].to_broadcast([128, w]),
                            in1=sid4[:, :w], op=ALU.is_equal)
                        e4[h] = e
                    return e4

                def oh_block(pan, jb, zsrc, e4):
                    r0 = (jb % GRP) * RB
                    obb = opool.tile([128, DCP, RB], BF16, tag="ob", name="ob")
                    for dc in range(DCP):
                        px = psX.tile([128, RB], F32, tag="px", name="px")
                        hs = halves[jb]
                        for i, h in enumerate(hs):
                            nc.tensor.matmul(
                                px[:], cn[pan][:, h, dc, :],
                                e4[h][:, r0:r0 + RB],
                                start=(i == 0), stop=(i == len(hs) - 1))
                        if pan == 0:
                            zbs = opool.tile([128, RB], BF16, tag="zbs",
                                             name="zbs")
                            nc.scalar.mul(zbs[:], zsrc(dc),
                                          s_fc[pan][:, dc:dc + 1])
                            nc.vector.tensor_add(obb[:, dc, :], zbs[:], px[:])
                        elif dc < 2:
                            nc.vector.scalar_tensor_tensor(
                                out=obb[:, dc, :], in0=zsrc(dc),
                                scalar=s_fc[pan][:, dc:dc + 1], in1=px[:],
                                op0=ALU.mult, op1=ALU.add)
                        else:
                            pxc = opool.tile([128, RB], BF16, tag="pxc",
                                             name="pxc")
                            nc.scalar.copy(pxc[:], px[:])
                            nc.vector.scalar_tensor_tensor(
                                out=obb[:, dc, :], in0=zsrc(dc),
                                scalar=s_fc[pan][:, dc:dc + 1], in1=pxc[:],
                                op0=ALU.mult, op1=ALU.add)
                    nc.sync.dma_start(outs[pan][jb], obb[:])

                def oh0_block(jb, e4):
                    zb = zbpool.tile([128, DCP, RB], BF16, tag="zb", name="zb")
                    nc.sync.dma_start(
                        zb[:], z0d[jb].rearrange("p (dc r) -> p dc r", dc=DCP))
                    oh_block(0, jb, lambda dc: zb[:, dc, :], e4)

                # --- emission schedule ---
                oh_done = 0
                e4_cur = None
                gsteps = gather_steps()
                gi = 0
                for ib in range(nblk):
                    p1_block(ib)
                    while gi < len(gsteps) and gi < (ib + 1) * 3:
                        gsteps[gi]()
                        gi += 1
                    if ib == 9:
                        assert gi == len(gsteps)
                        # x_m transposes + set path + stats #0
                        for h in range(2):
                            for kc in range(KC):
                                pt = psT.tile([128, 128], F32,
                                              tag=f"tr{kc % 2}", name="tr",
                                              bufs=1)
                                nc.tensor.transpose(
                                    pt[:], xm[h][:, kc * 128:(kc + 1) * 128],
                                    ident)
                                nc.scalar.copy(
                                    xmT[:, kc, h * 128:(h + 1) * 128], pt[:])
                        set_path(0, ws0)
                        pack_stats(0)
                    if ib == 13:
                        affine(0)
                    if ib >= 14 and oh_done < nblk - 8:
                        todo = min(2, nblk - 8 - oh_done)
                        for _ in range(todo):
                            if oh_done % GRP == 0:
                                e4_cur = oh_group_e(oh_done // GRP)
                            oh0_block(oh_done, e4_cur)
                            oh_done += 1

                # ---- tail: stats #1 while the last OH0 blocks drain ----
                set_path(1, ws1)
                pack_stats(1)
                while oh_done < nblk:
                    if oh_done % GRP == 0:
                        e4_cur = oh_group_e(oh_done // GRP)
                    oh0_block(oh_done, e4_cur)
                    oh_done += 1
                affine(1)
                for jb in range(nblk):
                    if jb % GRP == 0:
                        e4_cur = oh_group_e(jb // GRP)
                    if jb < Z1SP:
                        zb = zbpool.tile([128, DCP, RB], BF16, tag="zb",
                                         name="zb")
                        nc.sync.dma_start(
                            zb[:],
                            z1d[jb].rearrange("p (dc r) -> p dc r", dc=DCP))
                        oh_block(1, jb, lambda dc, _zb=zb: _zb[:, dc, :],
                                 e4_cur)
                    else:
                        oh_block(1, jb,
                                 lambda dc, _jb=jb: zres1[:, _jb - Z1SP, dc, :],
                                 e4_cur)

    nc.compile()
    return nc


def kernel(x, W_fc, b_fc, g_fc, be_fc, W_sh, b_sh, g_sh, be_sh, seg_ids,
           _want_trace=False):
    x = np.ascontiguousarray(np.asarray(x, dtype=np.float32))
    seg_ids = np.asarray(seg_ids, dtype=np.int32)
    counts, plans, max_rows, slots, halves = _plan(seg_ids)
    nblk = max_rows // RB

    key = (max_rows, slots, halves)
    if key not in _cache:
        _cache[key] = _build(max_rows, slots, halves)
    nc = _cache[key]

    import ml_dtypes
    io_np = ml_dtypes.bfloat16
    wfT = np.ascontiguousarray(np.asarray(W_fc, np.float32).T).astype(io_np)
    wsT = np.ascontiguousarray(np.asarray(W_sh, np.float32).T).astype(io_np)
    par = np.zeros((128, 162), dtype=np.float32)
    par[:, 0:8] = np.asarray(g_fc, np.float32).reshape(8, 128).T
    par[:, 8:16] = np.asarray(be_fc, np.float32).reshape(8, 128).T
    par[:, 16:24] = np.asarray(g_sh, np.float32).reshape(8, 128).T
    par[:, 24:32] = np.asarray(be_sh, np.float32).reshape(8, 128).T
    par[:, 32] = np.arange(128, dtype=np.float32)
    par[:, 33] = np.arange(128, 256, dtype=np.float32)
    par[:, 34:162] = np.eye(128, dtype=np.float32)

    in_maps = []
    for c in range(NC):
        p = plans[c]
        xp = np.zeros((max_rows, D), dtype=io_np)
        xp[p["col_ids"]] = x[p["grows"]].astype(io_np)
        xb = np.ascontiguousarray(
            xp.reshape(nblk, RB, KC, 128).transpose(0, 3, 2, 1))
        sid_row = np.full((1, max_rows), 999.0, dtype=io_np)
        sid_row[0, p["col_ids"]] = p["local_b"].astype(io_np)
        in_maps.append(dict(
            xB=xb, sid=sid_row, idx=p["idx"],
            wfT=wfT, wsT=wsT, par=par))

    kw = {}
    if _want_trace:
        kw = dict(trace=True)
    res = run_bass_kernel_spmd(nc, in_maps, core_ids=list(range(NC)), **kw)

    out = np.empty((N, D), dtype=np.float32)
    for c in range(NC):
        p = plans[c]
        for pan in range(NP):
            o = np.asarray(res.results[c][f"out{pan}"])
            o = o.transpose(0, 3, 2, 1).reshape(max_rows, PW)
            out[p["grows"], pan * PW:(pan + 1) * PW] = \
                o[p["col_ids"]].astype(np.float32)
    if _want_trace:
        return out, res
    return out
